# revision 20
# baseline (speedup 1.0000x reference)
"""RQSplineHead Trainium2 Bass kernel — scatter+scan version.

Math (per row, normalized x,y in [0,1]):
  params = softplus(h @ W.T + b) + 1e-4 -> w[9], hh[9], d[9]
  knots cxn = cumsum(w)/Sw; bin b: left knot a_b, width wn_b, height hn_b,
  derivs d_b, d_{b+1}; Dn = hn_b/wn_b; theta = (x - a_b)/wn_b:
    F(x) = cyn[b-1] + (HK_b*theta^2 + HD_b*theta) / (1 + SD_b*theta*(1-theta))
    HD_b = hn_b*d_b/Dn_b = d_b*wn_b,  HK_b = hn_b - HD_b,
    SD_b = (d_b+d_{b+1}-2Dn_b)/Dn_b = (d_b+d_{b+1})*wn_b/hn_b - 2
  out bin e = Ln(max(F(x_{e+1})-F(x_e), 1e-8))

Kernel structure per 128-row block:
  - 5 piecewise-constant chain images (A, RW, HK, HD, SD) built by planting
    per-bin jump values at knot edge positions E_j with ONE gpsimd
    local_scatter (fp32 carried as u16 pairs), then ONE gated
    tensor_tensor_scan expands all 5 sections.  A 6th scatter section plants
    the cyn jumps directly at output-bin positions (no scan; within-bin cyn
    terms cancel exactly).
  - knot-collision dedup on [P,G,8] tiles via an eq-gated mini-scan; dropped
    slots get index -1 (ignored by local_scatter).
  - per-edge math runs on [P,2,129] views (two row blocks per instruction)
    with custom fused DVE ops (the x grid is synthesized from the element
    index inside RQS_THETA); final Ln on ACT.
"""

import numpy as np
from contextlib import ExitStack

import concourse.bass as bass
import concourse.mybir as mybir
import concourse.tile as tile
from concourse import library_config
from concourse.masks import make_identity

f32 = mybir.dt.float32
i16 = mybir.dt.int16
u16 = mybir.dt.uint16
i32 = mybir.dt.int32
OP = mybir.AluOpType
AF = mybir.ActivationFunctionType

B_FULL = 131072
IN_DIM = 256
NE = 129          # edges
NB = 128          # output bins
NK = 9            # spline bins per row
ND = 8            # interior knots
ODIM = 27
N_CORES = 8
P = 128

NSEC = 5          # scanned chain sections: A, RW, HK, HD, SD
SECW = 130
DCYW = 128
IMGF = NSEC * SECW + DCYW + 2   # 780 f32
IMGU = IMGF * 2                 # 1560 u16
NV = NSEC + (NSEC + 1) * ND     # 53 values per row-block
BIG = 1000.0                    # dropped-slot offset


# ---------------- custom fused DVE ops ---------------------------------- #

def _ensure_dve_op(name, spec, subdim=False):
    import concourse.dve_ops as DO
    from concourse.dve_uop import DveOpSpec

    for op in DO.OPS:
        if op.name == name:
            return op
    opcode = DO._CUSTOM_DVE_ROW_BASE + len(DO.OPS)
    assert opcode < 0x20, "custom-DVE opcode rows exhausted"
    shas = {}
    for ver in ("v3", "v4"):
        try:
            uops = DO.lower(spec, ver=ver)
            shas[ver] = DveOpSpec(
                name=name, opcode=opcode, uops=uops,
                rd1_en=DO.has_src1(spec)).sha(ver)
        except Exception:
            pass
    assert shas, f"{name}: lower() failed for all DveVers"
    op = DO.DveOp(name, spec, subdim=subdim, uops_sha=shas)
    DO.OPS.append(op)
    DO._SUB_OPCODE_FOR_NAME[name] = opcode
    DO.CUSTOM_DVE_SPECS[name] = spec
    return op


def _make_ops():
    from concourse.dve_spec import (
        Spec, Src0, Src1, C0, C1, Idx, SubIdx, sq, maxx,
    )

    # n1 = theta^2 * HK
    sqmul = _ensure_dve_op(
        "RQS_SQMUL",
        Spec(body=sq(Src0) * Src1,
             reference=lambda in0, in1, s0, s1, imm2: in0 * in0 * in1))
    # D = 1 + SD*(theta - theta^2)   (s0 = 1.0)
    denom = _ensure_dve_op(
        "RQS_DENOM",
        Spec(body=Src1 * (Src0 - sq(Src0)) + C0,
             reference=lambda in0, in1, s0, s1, imm2:
                 in1 * (in0 - in0 * in0) + s0))
    # pt = max(dr + dcy, s0)
    addmax = _ensure_dve_op(
        "RQS_ADDMAX",
        Spec(body=maxx(Src0 + Src1, C0),
             reference=lambda in0, in1, s0, s1, imm2:
                 np.maximum(in0 + in1, s0)))

    # theta = x*RW - ARW with x generated from the element index:
    # x = Idx*s0 - SubIdx*s1 (s0 = 1/128, s1 = 129/128; subdim pages of 129)
    def _theta_ref(in0, in1, s0, s1, imm2):
        assert in0.ndim == 3
        pages, S = in0.shape[1], in0.shape[2]
        k = np.arange(pages * S, dtype=np.float32).reshape(1, pages, S)
        pg = (np.arange(pages, dtype=np.float32)
              .reshape(1, pages, 1) * np.ones((1, 1, S), np.float32))
        x = k * s0 - pg * s1
        return x * in0 - in1

    theta = _ensure_dve_op(
        "RQS_THETA",
        Spec(body=(Idx * C0 - SubIdx * C1) * Src0 - Src1,
             reference=_theta_ref),
        subdim=True)
    return sqmul, denom, addmax, theta


def build(ctx: ExitStack, tc: "tile.TileContext", h, W, b, out, rows,
          G=16, reps=1, scatter_dup=1, pair=2, ilv=1,
          es_bufs=3):
    nc = tc.nc
    nblk = rows // P
    nsb = nblk // G
    assert nsb * G == nblk
    sqmul, denom, addmax, theta = _make_ops()

    const = ctx.enter_context(tc.tile_pool(name="const", bufs=1))
    psum = ctx.enter_context(tc.tile_pool(name="psum", bufs=2, space="PSUM"))
    psum1 = ctx.enter_context(tc.tile_pool(name="psum1", bufs=1, space="PSUM"))
    hpool = ctx.enter_context(tc.tile_pool(name="hpool", bufs=3))
    bs = ctx.enter_context(tc.tile_pool(name="bs", bufs=2))
    es = ctx.enter_context(tc.tile_pool(name="es", bufs=es_bufs))
    outp = ctx.enter_context(tc.tile_pool(name="outp", bufs=3))

    # ---------------- constants ----------------
    ident = const.tile([P, P], f32)
    make_identity(nc, ident)

    gate9 = const.tile([P, G, NK], f32)
    nc.vector.memset(gate9, 1.0)
    nc.vector.memset(gate9[:, :, 0:1], 0.0)

    gate650 = const.tile([P, NSEC, SECW], f32)
    nc.vector.memset(gate650, 1.0)
    nc.vector.memset(gate650[:, :, 0:1], 0.0)

    # u16-index offsets for the 6 delta groups (5 chains + dcy):
    # idx_even = 2*psc + 2*(sect_off - BIG), idx_odd = idx_even + 1
    offs2e = const.tile([P, NSEC + 1, ND], f32)
    for q in range(NSEC):
        nc.vector.memset(offs2e[:, q], 2.0 * (q * SECW - BIG))
    nc.vector.memset(offs2e[:, NSEC], 2.0 * (NSEC * SECW - 1 - BIG))
    offs2o = const.tile([P, NSEC + 1, ND], f32)
    nc.vector.tensor_scalar(offs2o, offs2e, 1.0, None, op0=OP.add)
    # constant u16 idx pairs for the 5 section-base slots
    basei = const.tile([P, 2 * NSEC], i16)
    for q in range(NSEC):
        nc.vector.memset(basei[:, 2 * q:2 * q + 1], float(2 * q * SECW))
        nc.vector.memset(basei[:, 2 * q + 1:2 * q + 2],
                         float(2 * q * SECW + 1))

    ones1 = const.tile([1, P], f32)
    nc.vector.memset(ones1, 1.0)

    wraw = const.tile([P, IN_DIM], f32)
    nc.vector.memset(wraw, 0.0)
    nc.sync.dma_start(out=wraw[0:ODIM, :], in_=W)
    psw = psum1.tile([P, 2, P], f32)
    for k in range(2):
        nc.tensor.transpose(psw[:, k], wraw[:, k * P:(k + 1) * P], ident)
    wT = const.tile([P, 2, ODIM], f32)
    nc.scalar.copy(wT, psw[:, :, 0:ODIM])
    brow = const.tile([1, ODIM], f32)
    nc.sync.dma_start(out=brow, in_=b.rearrange("(o k) -> o k", o=1))

    nc.gpsimd.load_library(library_config.local_scatter)

    def body():
        for sb in range(nsb):
            # ---------- phase 1: params = softplus(h@W.T+b) ----------
            params = bs.tile([P, G, ODIM], f32, tag="params")
            for gg in range(G // 2):
                blk = sb * G + 2 * gg
                r0 = blk * P
                ht = hpool.tile([P, 2, IN_DIM], f32, tag="ht")
                nc.sync.dma_start(
                    out=ht, in_=h[r0:r0 + 2 * P].rearrange(
                        "(bb p) f -> p bb f", bb=2))
                for j in range(2):
                    psT = psum.tile([P, 2, P], f32, tag="psT")
                    for k in range(2):
                        nc.tensor.transpose(psT[:, k],
                                            ht[:, j, k * P:(k + 1) * P], ident)
                    hT = hpool.tile([P, 2, P], f32, tag="hT")
                    nc.scalar.copy(hT, psT)
                    pp = psum.tile([P, ODIM], f32, tag="pp")
                    nc.tensor.matmul(pp, hT[:, 0], wT[:, 0], start=True,
                                     stop=False)
                    nc.tensor.matmul(pp, hT[:, 1], wT[:, 1], start=False,
                                     stop=False)
                    nc.tensor.matmul(pp, ones1, brow, start=False, stop=True)
                    expt = hpool.tile([P, ODIM], f32, tag="expt")
                    nc.scalar.activation(expt, pp, AF.Exp)
                    nc.scalar.activation(params[:, 2 * gg + j], expt, AF.Ln,
                                         bias=1.0, scale=1.0)

            # ---------- phase 2: per-bin tables ----------
            w_in = bs.tile([P, G, NK], f32, tag="w_in")
            nc.vector.tensor_scalar(w_in, params[:, :, 0:NK], 1e-4, None,
                                    op0=OP.add)
            h_in = bs.tile([P, G, NK], f32, tag="h_in")
            nc.vector.tensor_scalar(h_in, params[:, :, NK:2 * NK], 1e-4, None,
                                    op0=OP.add)
            dpad = bs.tile([P, G, NK + 2], f32, tag="dpad")
            nc.vector.memset(dpad, 1.0)
            nc.vector.tensor_scalar(dpad[:, :, 1:NK + 1],
                                    params[:, :, 2 * NK:3 * NK],
                                    1e-4, None, op0=OP.add)

            cx = bs.tile([P, G, NK], f32, tag="cx")
            nc.vector.tensor_tensor_scan(
                cx.rearrange("p g k -> p (g k)"),
                gate9.rearrange("p g k -> p (g k)"),
                w_in.rearrange("p g k -> p (g k)"),
                0.0, op0=OP.mult, op1=OP.add)
            cy = bs.tile([P, G, NK], f32, tag="cy")
            nc.vector.tensor_tensor_scan(
                cy.rearrange("p g k -> p (g k)"),
                gate9.rearrange("p g k -> p (g k)"),
                h_in.rearrange("p g k -> p (g k)"),
                0.0, op0=OP.mult, op1=OP.add)

            rsw = bs.tile([P, G], f32, tag="rsw")
            nc.vector.reciprocal(rsw, cx[:, :, NK - 1])
            rsh = bs.tile([P, G], f32, tag="rsh")
            nc.vector.reciprocal(rsh, cy[:, :, NK - 1])
            rsw_b = rsw.unsqueeze(2).broadcast_to((P, G, NK))
            rsh_b = rsh.unsqueeze(2).broadcast_to((P, G, NK))

            cxn = bs.tile([P, G, NK], f32, tag="cxn")
            nc.vector.tensor_tensor(cxn, cx, rsw_b, OP.mult)

            e16 = bs.tile([P, G, ND], i16, tag="e16")
            nc.vector.tensor_scalar(e16, cxn[:, :, 0:ND], float(NB), 0.5,
                                    op0=OP.mult, op1=OP.add)

            eq8 = bs.tile([P, G, ND], f32, tag="eq8")
            nc.vector.memset(eq8[:, :, 0:1], 0.0)
            nc.vector.tensor_tensor(eq8[:, :, 1:ND], e16[:, :, 1:ND],
                                    e16[:, :, 0:ND - 1], OP.is_equal)
            last = bs.tile([P, G, ND], f32, tag="last")
            nc.vector.memset(last[:, :, ND - 1:ND], 1.0)
            nc.vector.tensor_scalar(last[:, :, 0:ND - 1], eq8[:, :, 1:ND],
                                    -1.0, 1.0, op0=OP.mult, op1=OP.add)
            psc = bs.tile([P, G, ND], f32, tag="psc")
            nc.vector.scalar_tensor_tensor(psc, e16, BIG, last, op0=OP.add,
                                           op1=OP.mult)

            idx = bs.tile([P, G, NV, 2], i16, tag="idx")
            nc.vector.tensor_scalar(
                idx[:, :, 0:NSEC, :].rearrange("p g a b -> p g (a b)"),
                basei.unsqueeze(1).broadcast_to((P, G, 2 * NSEC)),
                0.0, None, op0=OP.add)
            psc2 = bs.tile([P, G, ND], f32, tag="psc2")
            nc.vector.tensor_scalar(psc2, psc, 2.0, None, op0=OP.mult)
            psc2_b = psc2.unsqueeze(2).broadcast_to((P, G, NSEC + 1, ND))
            for bslot, offt in ((0, offs2e), (1, offs2o)):
                nc.vector.tensor_tensor(
                    idx[:, :, NSEC:NV, bslot].rearrange(
                        "p g (s k) -> p g s k", s=NSEC + 1),
                    psc2_b,
                    offt.unsqueeze(1).broadcast_to((P, G, NSEC + 1, ND)),
                    OP.add)

            # normalized per-bin tables; T4 rows = (RW, HK, HD, SD)
            wn9 = bs.tile([P, G, NK], f32, tag="wn9")
            nc.vector.tensor_tensor(wn9, w_in, rsw_b, OP.mult)
            hn9 = bs.tile([P, G, NK], f32, tag="hn9")
            nc.vector.tensor_tensor(hn9, h_in, rsh_b, OP.mult)
            t4 = bs.tile([P, G, 4, NK], f32, tag="t4")
            RWt, HKt, HDt, SDt = (t4[:, :, 0], t4[:, :, 1], t4[:, :, 2],
                                  t4[:, :, 3])
            nc.vector.reciprocal(RWt, wn9)
            nc.vector.tensor_tensor(HDt, dpad[:, :, 0:NK], wn9, OP.mult)
            nc.vector.tensor_tensor(HKt, hn9, HDt, OP.subtract)
            rhn = bs.tile([P, G, NK], f32, tag="rhn")
            nc.vector.reciprocal(rhn, hn9)
            s1 = bs.tile([P, G, NK], f32, tag="s1")
            nc.vector.tensor_tensor(s1, dpad[:, :, 0:NK], dpad[:, :, 1:NK + 1],
                                    OP.add)
            nc.vector.tensor_tensor(s1, s1, wn9, OP.mult)
            nc.vector.tensor_tensor(s1, s1, rhn, OP.mult)
            nc.vector.tensor_scalar(SDt, s1, -2.0, None, op0=OP.add)
            # ARW = a_b / wn_b (left knot * RW); ARW_0 = 0
            arw = bs.tile([P, G, NK], f32, tag="arw")
            nc.vector.memset(arw[:, :, 0:1], 0.0)
            nc.vector.tensor_tensor(arw[:, :, 1:NK], cxn[:, :, 0:ND],
                                    RWt[:, :, 1:NK], OP.mult)

            v53 = bs.tile([P, G, NV], f32, tag="v53")
            nc.vector.memset(v53[:, :, 0:1], 0.0)
            nc.vector.tensor_scalar(v53[:, :, 1:NSEC], t4[:, :, :, 0],
                                    0.0, None, op0=OP.add)
            nc.vector.tensor_tensor(v53[:, :, 5:13], arw[:, :, 1:NK],
                                    arw[:, :, 0:ND], OP.subtract)
            nc.vector.tensor_tensor(
                v53[:, :, 13:45].rearrange("p g (s k) -> p g s k", s=4),
                t4[:, :, :, 1:NK], t4[:, :, :, 0:ND], OP.subtract)
            nc.vector.tensor_scalar(v53[:, :, 45:53], hn9[:, :, 0:ND], 0.0,
                                    None, op0=OP.add)

            eqg = bs.tile([P, G, NV], f32, tag="eqg")
            nc.vector.memset(eqg[:, :, 0:NSEC], 0.0)
            nc.vector.tensor_scalar(
                eqg[:, :, NSEC:NV].rearrange("p g (s k) -> p g s k",
                                             s=NSEC + 1),
                eq8.unsqueeze(2).broadcast_to((P, G, NSEC + 1, ND)),
                0.0, None, op0=OP.add)
            d53 = bs.tile([P, G, NV], f32, tag="d53")
            nc.vector.tensor_tensor_scan(
                d53.rearrange("p g k -> p (g k)"),
                eqg.rearrange("p g k -> p (g k)"),
                v53.rearrange("p g k -> p (g k)"),
                0.0, op0=OP.mult, op1=OP.add)

            # ---------- phase 3: per-pair edge evaluation ----------
            # `ilv` pair-groups are emitted step-interleaved so each DVE op
            # hides the producer->consumer semaphore latency of the other.
            PR = pair
            NPAIR = G // PR
            for q0 in range(0, NPAIR, ilv):
                grp = list(range(q0, min(q0 + ilv, NPAIR)))
                T = {}
                for gg in grp:
                    sl = gg - q0
                    img = es.tile([P, PR, IMGU], u16, tag=f"img{sl}")
                    for j in range(PR):
                        for _rep in range(scatter_dup):
                            nc.gpsimd.local_scatter(
                                out_ap=img[:, j],
                                data_ap=d53[:, PR * gg + j].bitcast(u16),
                                idxs_ap=idx[:, PR * gg + j].rearrange(
                                    "p a b -> p (a b)"),
                                channels=P,
                                num_elems=IMGU,
                                num_idxs=2 * NV,
                            )
                    T[gg, "img"] = img
                for gg in grp:
                    sl = gg - q0
                    img = T[gg, "img"]
                    imgf = img.bitcast(f32)
                    ch = es.tile([P, PR, NSEC * SECW], f32, tag=f"ch{sl}")
                    for j in range(PR):
                        nc.vector.tensor_tensor_scan(
                            ch[:, j],
                            gate650.rearrange("p a b -> p (a b)"),
                            imgf[:, j, 0:NSEC * SECW],
                            0.0, op0=OP.mult, op1=OP.add)
                    chv = ch.rearrange("p bb (s e) -> p bb s e", s=NSEC)
                    T[gg, "chv"] = chv
                    T[gg, "dcy"] = imgf[:, :, NSEC * SECW:NSEC * SECW + DCYW]
                for gg in grp:
                    sl = gg - q0
                    chv = T[gg, "chv"]
                    th = es.tile([P, PR, NE], f32, tag=f"th{sl}")
                    nc.vector._custom_dve(theta, out=th,
                                          in0=chv[:, :, 1, 0:NE],
                                          in1=chv[:, :, 0, 0:NE],
                                          s0=1.0 / NB, s1=float(NE) / NB)
                    T[gg, "th"] = th
                for gg in grp:
                    sl = gg - q0
                    n1 = es.tile([P, PR, NE], f32, tag=f"n1{sl}")
                    nc.vector._custom_dve(sqmul, out=n1, in0=T[gg, "th"],
                                          in1=T[gg, "chv"][:, :, 2, 0:NE])
                    T[gg, "n1"] = n1
                for gg in grp:
                    sl = gg - q0
                    n2 = es.tile([P, PR, NE], f32, tag=f"n2{sl}")
                    nc.vector.tensor_tensor(n2, T[gg, "chv"][:, :, 3, 0:NE],
                                            T[gg, "th"], OP.mult)
                    T[gg, "n2"] = n2
                for gg in grp:
                    nc.vector.tensor_tensor(T[gg, "n1"], T[gg, "n1"],
                                            T[gg, "n2"], OP.add)
                for gg in grp:
                    sl = gg - q0
                    dd = es.tile([P, PR, NE], f32, tag=f"dd{sl}")
                    nc.vector._custom_dve(denom, out=dd, in0=T[gg, "th"],
                                          in1=T[gg, "chv"][:, :, 4, 0:NE],
                                          s0=1.0)
                    T[gg, "dd"] = dd
                for gg in grp:
                    sl = gg - q0
                    rd = es.tile([P, PR, NE], f32, tag=f"rd{sl}")
                    nc.vector.reciprocal_approx_fast(rd, T[gg, "dd"])
                    T[gg, "rd"] = rd
                for gg in grp:
                    sl = gg - q0
                    rat = es.tile([P, PR, NE], f32, tag=f"rat{sl}")
                    nc.vector.tensor_tensor(rat, T[gg, "n1"], T[gg, "rd"],
                                            OP.mult)
                    T[gg, "rat"] = rat
                for gg in grp:
                    sl = gg - q0
                    rat = T[gg, "rat"]
                    dr = outp.tile([P, PR, NB], f32, tag=f"dr{sl}")
                    nc.vector.tensor_tensor(dr, rat[:, :, 1:NE],
                                            rat[:, :, 0:NB], OP.subtract)
                    T[gg, "dr"] = dr
                for gg in grp:
                    sl = gg - q0
                    pt = outp.tile([P, PR, NB], f32, tag=f"pt{sl}")
                    nc.vector._custom_dve(addmax, out=pt, in0=T[gg, "dr"],
                                          in1=T[gg, "dcy"], s0=1e-8)
                    T[gg, "pt"] = pt
                for gg in grp:
                    sl = gg - q0
                    blk = sb * G + PR * gg
                    r0 = blk * P
                    ot = outp.tile([P, PR, NB], f32, tag=f"ot{sl}")
                    nc.scalar.activation(ot, T[gg, "pt"], AF.Ln)
                    nc.sync.dma_start(
                        out=out[r0:r0 + PR * P].rearrange(
                            "(bb p) c -> p bb c", bb=PR),
                        in_=ot)

    if reps == 1:
        body()
    else:
        with tc.For_i(0, reps):
            body()


def make_nc(rows, G=16, reps=1, scatter_dup=1, pair=2, ilv=1,
            es_bufs=3):
    import concourse.bacc as bacc
    from concourse.hw_specs import get_activation_tables

    class _Bacc(bacc.Bacc):
        """Force all activations onto the exp+ln combined table set so the
        compiler never inserts per-switch table reloads."""

        def insert_act_table_loads(self):
            import bass_rust as _bass_rust
            has_activation = any(
                isinstance(i, mybir.InstActivation)
                for blk in self.main_func.blocks
                for i in blk.instructions
            )
            if not has_activation:
                return
            tables = []
            for name, funcs in get_activation_tables(self.m.arch).items():
                if name == "natural_log_exp_and_others":
                    tables.append((name, funcs))
                else:
                    tables.append((name, set()))
            _bass_rust.insert_act_table_loads(self, tables)

    nc = _Bacc("TRN2", target_bir_lowering=False, debug=False,
               num_devices=N_CORES)
    h_t = nc.dram_tensor("h", [rows, IN_DIM], f32, kind="ExternalInput").ap()
    W_t = nc.dram_tensor("W", [ODIM, IN_DIM], f32, kind="ExternalInput").ap()
    b_t = nc.dram_tensor("b", [ODIM], f32, kind="ExternalInput").ap()
    out_t = nc.dram_tensor("out", [rows, NB], f32, kind="ExternalOutput").ap()
    with tile.TileContext(nc) as tc:
        with ExitStack() as ctx:
            build(ctx, tc, h_t, W_t, b_t, out_t, rows, G=G, reps=reps,
                  scatter_dup=scatter_dup, pair=pair, ilv=ilv,
                  es_bufs=es_bufs)
    nc.compile()
    return nc


_cache = {}


def kernel(h, W, b):
    h = np.ascontiguousarray(h, dtype=np.float32)
    W = np.ascontiguousarray(W, dtype=np.float32)
    b = np.ascontiguousarray(b, dtype=np.float32)
    rows = h.shape[0] // N_CORES
    key = ("nc", rows)
    if key not in _cache:
        _cache[key] = make_nc(rows)
    nc = _cache[key]
    from concourse.bass_utils import run_bass_kernel_spmd
    in_maps = [
        {"h": h[i * rows:(i + 1) * rows], "W": W, "b": b}
        for i in range(N_CORES)
    ]
    res = run_bass_kernel_spmd(nc, in_maps, core_ids=list(range(N_CORES)))
    return np.concatenate([r["out"] for r in res.results], axis=0)


if __name__ == "__main__":
    rng = np.random.default_rng(0)
    h = rng.standard_normal((B_FULL, IN_DIM), dtype=np.float32)
    W = (rng.standard_normal((ODIM, IN_DIM), dtype=np.float32) / 16.0)
    b = rng.standard_normal((ODIM,), dtype=np.float32) * 0.01
    out = kernel(h, W, b)
    print(out.shape, out.dtype, out[:2, :4])


# revision 24
# speedup vs baseline: 1.2614x; 1.2614x over previous
"""RQSplineHead Trainium2 Bass kernel — scatter+scan version.

Math (per row, normalized x,y in [0,1]):
  params = softplus(h @ W.T + b) + 1e-4 -> w[9], hh[9], d[9]
  knots cxn = cumsum(w)/Sw; bin b: left knot a_b, width wn_b, height hn_b,
  derivs d_b, d_{b+1}; Dn = hn_b/wn_b; theta = (x - a_b)/wn_b:
    F(x) = cyn[b-1] + (HK_b*theta^2 + HD_b*theta) / (1 + SD_b*theta*(1-theta))
    HD_b = hn_b*d_b/Dn_b = d_b*wn_b,  HK_b = hn_b - HD_b,
    SD_b = (d_b+d_{b+1}-2Dn_b)/Dn_b = (d_b+d_{b+1})*wn_b/hn_b - 2
  out bin e = Ln(max(F(x_{e+1})-F(x_e), 1e-8))

Kernel structure per 128-row block:
  - 5 piecewise-constant chain images (A, RW, HK, HD, SD) built by planting
    per-bin jump values at knot edge positions E_j with ONE gpsimd
    local_scatter (fp32 carried as u16 pairs), then ONE gated
    tensor_tensor_scan expands all 5 sections.  A 6th scatter section plants
    the cyn jumps directly at output-bin positions (no scan; within-bin cyn
    terms cancel exactly).
  - knot-collision dedup on [P,G,8] tiles via an eq-gated mini-scan; dropped
    slots get index -1 (ignored by local_scatter).
  - per-edge math runs on [P,2,129] views (two row blocks per instruction)
    with custom fused DVE ops (the x grid is synthesized from the element
    index inside RQS_THETA); final Ln on ACT.
"""

import numpy as np
from contextlib import ExitStack

import concourse.bass as bass
import concourse.mybir as mybir
import concourse.tile as tile
from concourse import library_config
from concourse.masks import make_identity

f32 = mybir.dt.float32
i16 = mybir.dt.int16
u16 = mybir.dt.uint16
i32 = mybir.dt.int32
OP = mybir.AluOpType
AF = mybir.ActivationFunctionType

B_FULL = 131072
IN_DIM = 256
NE = 129          # edges
NB = 128          # output bins
NK = 9            # spline bins per row
ND = 8            # interior knots
ODIM = 27
N_CORES = 8
P = 128

NSEC = 5          # scanned chain sections: A, RW, HK, HD, SD
SECW = 130
DCYW = 128
IMGF = NSEC * SECW + DCYW + 2   # 780 f32
IMGU = IMGF * 2                 # 1560 u16
NV = NSEC + (NSEC + 1) * ND     # 53 values per row-block
BIG = 1000.0                    # dropped-slot offset


# ---------------- custom fused DVE ops ---------------------------------- #

def _ensure_dve_op(name, spec, subdim=False):
    import concourse.dve_ops as DO
    from concourse.dve_uop import DveOpSpec

    for op in DO.OPS:
        if op.name == name:
            return op
    opcode = DO._CUSTOM_DVE_ROW_BASE + len(DO.OPS)
    assert opcode < 0x20, "custom-DVE opcode rows exhausted"
    shas = {}
    for ver in ("v3", "v4"):
        try:
            uops = DO.lower(spec, ver=ver)
            shas[ver] = DveOpSpec(
                name=name, opcode=opcode, uops=uops,
                rd1_en=DO.has_src1(spec)).sha(ver)
        except Exception:
            pass
    assert shas, f"{name}: lower() failed for all DveVers"
    op = DO.DveOp(name, spec, subdim=subdim, uops_sha=shas)
    DO.OPS.append(op)
    DO._SUB_OPCODE_FOR_NAME[name] = opcode
    DO.CUSTOM_DVE_SPECS[name] = spec
    return op


def _make_ops():
    from concourse.dve_spec import (
        Spec, Src0, Src1, C0, C1, Idx, SubIdx, sq, maxx,
    )

    # n1 = theta^2 * HK
    sqmul = _ensure_dve_op(
        "RQS_SQMUL",
        Spec(body=sq(Src0) * Src1,
             reference=lambda in0, in1, s0, s1, imm2: in0 * in0 * in1))
    # D = 1 + SD*(theta - theta^2)   (s0 = 1.0)
    denom = _ensure_dve_op(
        "RQS_DENOM",
        Spec(body=Src1 * (Src0 - sq(Src0)) + C0,
             reference=lambda in0, in1, s0, s1, imm2:
                 in1 * (in0 - in0 * in0) + s0))
    # pt = max(dr + dcy, s0)
    addmax = _ensure_dve_op(
        "RQS_ADDMAX",
        Spec(body=maxx(Src0 + Src1, C0),
             reference=lambda in0, in1, s0, s1, imm2:
                 np.maximum(in0 + in1, s0)))

    # theta = x*RW - ARW with x generated from the element index:
    # x = Idx*s0 - SubIdx*s1 (s0 = 1/128, s1 = 129/128; subdim pages of 129)
    def _theta_ref(in0, in1, s0, s1, imm2):
        assert in0.ndim == 3
        pages, S = in0.shape[1], in0.shape[2]
        k = np.arange(pages * S, dtype=np.float32).reshape(1, pages, S)
        pg = (np.arange(pages, dtype=np.float32)
              .reshape(1, pages, 1) * np.ones((1, 1, S), np.float32))
        x = k * s0 - pg * s1
        return x * in0 - in1

    theta = _ensure_dve_op(
        "RQS_THETA",
        Spec(body=(Idx * C0 - SubIdx * C1) * Src0 - Src1,
             reference=_theta_ref),
        subdim=True)
    return sqmul, denom, addmax, theta


def build(ctx: ExitStack, tc: "tile.TileContext", h, W, b, out, rows,
          G=16, reps=1, scatter_dup=1, pair=2, ilv=1,
          es_bufs=3):
    nc = tc.nc
    nblk = rows // P
    nsb = nblk // G
    assert nsb * G == nblk
    sqmul, denom, addmax, theta = _make_ops()

    const = ctx.enter_context(tc.tile_pool(name="const", bufs=1))
    psum = ctx.enter_context(tc.tile_pool(name="psum", bufs=2, space="PSUM"))
    psum1 = ctx.enter_context(tc.tile_pool(name="psum1", bufs=1, space="PSUM"))
    hpool = ctx.enter_context(tc.tile_pool(name="hpool", bufs=3))
    bs = ctx.enter_context(tc.tile_pool(name="bs", bufs=2))
    es = ctx.enter_context(tc.tile_pool(name="es", bufs=es_bufs))
    outp = ctx.enter_context(tc.tile_pool(name="outp", bufs=3))

    # ---------------- constants ----------------
    ident = const.tile([P, P], f32)
    make_identity(nc, ident)

    gate9 = const.tile([P, G, NK], f32)
    nc.vector.memset(gate9, 1.0)
    nc.vector.memset(gate9[:, :, 0:1], 0.0)

    gate650 = const.tile([P, NSEC, SECW], f32)
    nc.vector.memset(gate650, 1.0)
    nc.vector.memset(gate650[:, :, 0:1], 0.0)

    # u16-index offsets for the 6 delta groups (5 chains + dcy):
    # idx_even = 2*psc + 2*(sect_off - BIG), idx_odd = idx_even + 1
    offs2e = const.tile([P, NSEC + 1, ND], f32)
    for q in range(NSEC):
        nc.vector.memset(offs2e[:, q], 2.0 * (q * SECW - BIG))
    nc.vector.memset(offs2e[:, NSEC], 2.0 * (NSEC * SECW - 1 - BIG))
    offs2o = const.tile([P, NSEC + 1, ND], f32)
    nc.vector.tensor_scalar(offs2o, offs2e, 1.0, None, op0=OP.add)
    # constant u16 idx pairs for the 5 section-base slots
    basei = const.tile([P, 2 * NSEC], i16)
    for q in range(NSEC):
        nc.vector.memset(basei[:, 2 * q:2 * q + 1], float(2 * q * SECW))
        nc.vector.memset(basei[:, 2 * q + 1:2 * q + 2],
                         float(2 * q * SECW + 1))

    ones1 = const.tile([1, P], f32)
    nc.vector.memset(ones1, 1.0)

    wraw = const.tile([P, IN_DIM], f32)
    nc.vector.memset(wraw, 0.0)
    nc.sync.dma_start(out=wraw[0:ODIM, :], in_=W)
    psw = psum1.tile([P, 2, P], f32)
    for k in range(2):
        nc.tensor.transpose(psw[:, k], wraw[:, k * P:(k + 1) * P], ident)
    wT = const.tile([P, 2, ODIM], f32)
    nc.scalar.copy(wT, psw[:, :, 0:ODIM])
    brow = const.tile([1, ODIM], f32)
    nc.sync.dma_start(out=brow, in_=b.rearrange("(o k) -> o k", o=1))

    nc.gpsimd.load_library(library_config.local_scatter)

    def body():
        for sb in range(nsb):
            # ---------- phase 1: params = softplus(h@W.T+b) ----------
            params = bs.tile([P, G, ODIM], f32, tag="params")
            for gg in range(G // 2):
                blk = sb * G + 2 * gg
                r0 = blk * P
                ht = hpool.tile([P, 2, IN_DIM], f32, tag="ht")
                nc.sync.dma_start(
                    out=ht, in_=h[r0:r0 + 2 * P].rearrange(
                        "(bb p) f -> p bb f", bb=2))
                for j in range(2):
                    psT = psum.tile([P, 2, P], f32, tag="psT")
                    for k in range(2):
                        nc.tensor.transpose(psT[:, k],
                                            ht[:, j, k * P:(k + 1) * P], ident)
                    hT = hpool.tile([P, 2, P], f32, tag="hT")
                    nc.scalar.copy(hT, psT)
                    pp = psum.tile([P, ODIM], f32, tag="pp")
                    nc.tensor.matmul(pp, hT[:, 0], wT[:, 0], start=True,
                                     stop=False)
                    nc.tensor.matmul(pp, hT[:, 1], wT[:, 1], start=False,
                                     stop=False)
                    nc.tensor.matmul(pp, ones1, brow, start=False, stop=True)
                    expt = hpool.tile([P, ODIM], f32, tag="expt")
                    nc.scalar.activation(expt, pp, AF.Exp)
                    nc.scalar.activation(params[:, 2 * gg + j], expt, AF.Ln,
                                         bias=1.0, scale=1.0)

            # ---------- phase 2: per-bin tables ----------
            w_in = bs.tile([P, G, NK], f32, tag="w_in")
            nc.vector.tensor_scalar(w_in, params[:, :, 0:NK], 1e-4, None,
                                    op0=OP.add)
            h_in = bs.tile([P, G, NK], f32, tag="h_in")
            nc.vector.tensor_scalar(h_in, params[:, :, NK:2 * NK], 1e-4, None,
                                    op0=OP.add)
            dpad = bs.tile([P, G, NK + 2], f32, tag="dpad")
            nc.vector.memset(dpad, 1.0)
            nc.vector.tensor_scalar(dpad[:, :, 1:NK + 1],
                                    params[:, :, 2 * NK:3 * NK],
                                    1e-4, None, op0=OP.add)

            cx = bs.tile([P, G, NK], f32, tag="cx")
            nc.vector.tensor_tensor_scan(
                cx.rearrange("p g k -> p (g k)"),
                gate9.rearrange("p g k -> p (g k)"),
                w_in.rearrange("p g k -> p (g k)"),
                0.0, op0=OP.mult, op1=OP.add)
            cy = bs.tile([P, G, NK], f32, tag="cy")
            nc.vector.tensor_tensor_scan(
                cy.rearrange("p g k -> p (g k)"),
                gate9.rearrange("p g k -> p (g k)"),
                h_in.rearrange("p g k -> p (g k)"),
                0.0, op0=OP.mult, op1=OP.add)

            rsw = bs.tile([P, G], f32, tag="rsw")
            nc.vector.reciprocal(rsw, cx[:, :, NK - 1])
            rsh = bs.tile([P, G], f32, tag="rsh")
            nc.vector.reciprocal(rsh, cy[:, :, NK - 1])
            rsw_b = rsw.unsqueeze(2).broadcast_to((P, G, NK))
            rsh_b = rsh.unsqueeze(2).broadcast_to((P, G, NK))

            cxn = bs.tile([P, G, NK], f32, tag="cxn")
            nc.vector.tensor_tensor(cxn, cx, rsw_b, OP.mult)

            e16 = bs.tile([P, G, ND], i16, tag="e16")
            nc.vector.tensor_scalar(e16, cxn[:, :, 0:ND], float(NB), 0.5,
                                    op0=OP.mult, op1=OP.add)

            eq8 = bs.tile([P, G, ND], f32, tag="eq8")
            nc.vector.memset(eq8[:, :, 0:1], 0.0)
            nc.vector.tensor_tensor(eq8[:, :, 1:ND], e16[:, :, 1:ND],
                                    e16[:, :, 0:ND - 1], OP.is_equal)
            last = bs.tile([P, G, ND], f32, tag="last")
            nc.vector.memset(last[:, :, ND - 1:ND], 1.0)
            nc.vector.tensor_scalar(last[:, :, 0:ND - 1], eq8[:, :, 1:ND],
                                    -1.0, 1.0, op0=OP.mult, op1=OP.add)
            psc = bs.tile([P, G, ND], f32, tag="psc")
            nc.vector.scalar_tensor_tensor(psc, e16, BIG, last, op0=OP.add,
                                           op1=OP.mult)

            idx = bs.tile([P, G, NV, 2], i16, tag="idx")
            nc.vector.tensor_scalar(
                idx[:, :, 0:NSEC, :].rearrange("p g a b -> p g (a b)"),
                basei.unsqueeze(1).broadcast_to((P, G, 2 * NSEC)),
                0.0, None, op0=OP.add)
            psc2 = bs.tile([P, G, ND], f32, tag="psc2")
            nc.vector.tensor_scalar(psc2, psc, 2.0, None, op0=OP.mult)
            psc2_b = psc2.unsqueeze(2).broadcast_to((P, G, NSEC + 1, ND))
            for bslot, offt in ((0, offs2e), (1, offs2o)):
                nc.vector.tensor_tensor(
                    idx[:, :, NSEC:NV, bslot].rearrange(
                        "p g (s k) -> p g s k", s=NSEC + 1),
                    psc2_b,
                    offt.unsqueeze(1).broadcast_to((P, G, NSEC + 1, ND)),
                    OP.add)

            # normalized per-bin tables; T4 rows = (RW, HK, HD, SD)
            wn9 = bs.tile([P, G, NK], f32, tag="wn9")
            nc.vector.tensor_tensor(wn9, w_in, rsw_b, OP.mult)
            hn9 = bs.tile([P, G, NK], f32, tag="hn9")
            nc.vector.tensor_tensor(hn9, h_in, rsh_b, OP.mult)
            t4 = bs.tile([P, G, 4, NK], f32, tag="t4")
            RWt, HKt, HDt, SDt = (t4[:, :, 0], t4[:, :, 1], t4[:, :, 2],
                                  t4[:, :, 3])
            nc.vector.reciprocal(RWt, wn9)
            nc.vector.tensor_tensor(HDt, dpad[:, :, 0:NK], wn9, OP.mult)
            nc.vector.tensor_tensor(HKt, hn9, HDt, OP.subtract)
            rhn = bs.tile([P, G, NK], f32, tag="rhn")
            nc.vector.reciprocal(rhn, hn9)
            s1 = bs.tile([P, G, NK], f32, tag="s1")
            nc.vector.tensor_tensor(s1, dpad[:, :, 0:NK], dpad[:, :, 1:NK + 1],
                                    OP.add)
            nc.vector.tensor_tensor(s1, s1, wn9, OP.mult)
            nc.vector.tensor_tensor(s1, s1, rhn, OP.mult)
            nc.vector.tensor_scalar(SDt, s1, -2.0, None, op0=OP.add)
            # ARW = a_b / wn_b (left knot * RW); ARW_0 = 0
            arw = bs.tile([P, G, NK], f32, tag="arw")
            nc.vector.memset(arw[:, :, 0:1], 0.0)
            nc.vector.tensor_tensor(arw[:, :, 1:NK], cxn[:, :, 0:ND],
                                    RWt[:, :, 1:NK], OP.mult)

            v53 = bs.tile([P, G, NV], f32, tag="v53")
            nc.vector.memset(v53[:, :, 0:1], 0.0)
            nc.vector.tensor_scalar(v53[:, :, 1:NSEC], t4[:, :, :, 0],
                                    0.0, None, op0=OP.add)
            nc.vector.tensor_tensor(v53[:, :, 5:13], arw[:, :, 1:NK],
                                    arw[:, :, 0:ND], OP.subtract)
            nc.vector.tensor_tensor(
                v53[:, :, 13:45].rearrange("p g (s k) -> p g s k", s=4),
                t4[:, :, :, 1:NK], t4[:, :, :, 0:ND], OP.subtract)
            nc.vector.tensor_scalar(v53[:, :, 45:53], hn9[:, :, 0:ND], 0.0,
                                    None, op0=OP.add)

            eqg = bs.tile([P, G, NV], f32, tag="eqg")
            nc.vector.memset(eqg[:, :, 0:NSEC], 0.0)
            nc.vector.tensor_scalar(
                eqg[:, :, NSEC:NV].rearrange("p g (s k) -> p g s k",
                                             s=NSEC + 1),
                eq8.unsqueeze(2).broadcast_to((P, G, NSEC + 1, ND)),
                0.0, None, op0=OP.add)
            d53 = bs.tile([P, G, NV], f32, tag="d53")
            nc.vector.tensor_tensor_scan(
                d53.rearrange("p g k -> p (g k)"),
                eqg.rearrange("p g k -> p (g k)"),
                v53.rearrange("p g k -> p (g k)"),
                0.0, op0=OP.mult, op1=OP.add)

            # ---------- phase 3: per-pair edge evaluation ----------
            # `ilv` pair-groups are emitted step-interleaved so each DVE op
            # hides the producer->consumer semaphore latency of the other.
            PR = pair
            NPAIR = G // PR
            for q0 in range(0, NPAIR, ilv):
                grp = list(range(q0, min(q0 + ilv, NPAIR)))
                T = {}
                for gg in grp:
                    sl = gg - q0
                    img = es.tile([P, PR, IMGU], u16, tag=f"img{sl}")
                    for j in range(PR):
                        for _rep in range(scatter_dup):
                            nc.gpsimd.local_scatter(
                                out_ap=img[:, j],
                                data_ap=d53[:, PR * gg + j].bitcast(u16),
                                idxs_ap=idx[:, PR * gg + j].rearrange(
                                    "p a b -> p (a b)"),
                                channels=P,
                                num_elems=IMGU,
                                num_idxs=2 * NV,
                            )
                    T[gg, "img"] = img
                for gg in grp:
                    sl = gg - q0
                    img = T[gg, "img"]
                    imgf = img.bitcast(f32)
                    ch = es.tile([P, PR, NSEC * SECW], f32, tag=f"ch{sl}")
                    for j in range(PR):
                        nc.vector.tensor_tensor_scan(
                            ch[:, j],
                            gate650.rearrange("p a b -> p (a b)"),
                            imgf[:, j, 0:NSEC * SECW],
                            0.0, op0=OP.mult, op1=OP.add)
                    chv = ch.rearrange("p bb (s e) -> p bb s e", s=NSEC)
                    T[gg, "chv"] = chv
                    T[gg, "dcy"] = imgf[:, :, NSEC * SECW:NSEC * SECW + DCYW]
                for gg in grp:
                    sl = gg - q0
                    chv = T[gg, "chv"]
                    th = es.tile([P, PR, NE], f32, tag=f"th{sl}")
                    nc.vector._custom_dve(theta, out=th,
                                          in0=chv[:, :, 1, 0:NE],
                                          in1=chv[:, :, 0, 0:NE],
                                          s0=1.0 / NB, s1=float(NE) / NB)
                    T[gg, "th"] = th
                for gg in grp:
                    sl = gg - q0
                    n1 = es.tile([P, PR, NE], f32, tag=f"n1{sl}")
                    nc.vector._custom_dve(sqmul, out=n1, in0=T[gg, "th"],
                                          in1=T[gg, "chv"][:, :, 2, 0:NE])
                    T[gg, "n1"] = n1
                for gg in grp:
                    sl = gg - q0
                    n2 = es.tile([P, PR, NE], f32, tag=f"n2{sl}")
                    nc.vector.tensor_tensor(n2, T[gg, "chv"][:, :, 3, 0:NE],
                                            T[gg, "th"], OP.mult)
                    T[gg, "n2"] = n2
                for gg in grp:
                    nc.vector.tensor_tensor(T[gg, "n1"], T[gg, "n1"],
                                            T[gg, "n2"], OP.add)
                for gg in grp:
                    sl = gg - q0
                    dd = es.tile([P, PR, NE], f32, tag=f"dd{sl}")
                    nc.vector._custom_dve(denom, out=dd, in0=T[gg, "th"],
                                          in1=T[gg, "chv"][:, :, 4, 0:NE],
                                          s0=1.0)
                    T[gg, "dd"] = dd
                for gg in grp:
                    sl = gg - q0
                    rd = es.tile([P, PR, NE], f32, tag=f"rd{sl}")
                    nc.vector.reciprocal_approx_fast(rd, T[gg, "dd"])
                    T[gg, "rd"] = rd
                for gg in grp:
                    sl = gg - q0
                    rat = es.tile([P, PR, NE], f32, tag=f"rat{sl}")
                    nc.vector.tensor_tensor(rat, T[gg, "n1"], T[gg, "rd"],
                                            OP.mult)
                    T[gg, "rat"] = rat
                for gg in grp:
                    sl = gg - q0
                    rat = T[gg, "rat"]
                    dr = outp.tile([P, PR, NB], f32, tag=f"dr{sl}")
                    nc.vector.tensor_tensor(dr, rat[:, :, 1:NE],
                                            rat[:, :, 0:NB], OP.subtract)
                    T[gg, "dr"] = dr
                for gg in grp:
                    sl = gg - q0
                    pt = outp.tile([P, PR, NB], f32, tag=f"pt{sl}")
                    nc.vector._custom_dve(addmax, out=pt, in0=T[gg, "dr"],
                                          in1=T[gg, "dcy"], s0=1e-8)
                    T[gg, "pt"] = pt
                for gg in grp:
                    sl = gg - q0
                    blk = sb * G + PR * gg
                    r0 = blk * P
                    ot = outp.tile([P, PR, NB], f32, tag=f"ot{sl}")
                    nc.scalar.activation(ot, T[gg, "pt"], AF.Ln)
                    nc.sync.dma_start(
                        out=out[r0:r0 + PR * P].rearrange(
                            "(bb p) c -> p bb c", bb=PR),
                        in_=ot)

    if reps == 1:
        body()
    else:
        with tc.For_i(0, reps):
            body()


def make_nc(rows, G=16, reps=1, scatter_dup=1, pair=2, ilv=1,
            es_bufs=3):
    import concourse.bacc as bacc
    from concourse.hw_specs import get_activation_tables

    class _Bacc(bacc.Bacc):
        """Force all activations onto the exp+ln combined table set so the
        compiler never inserts per-switch table reloads."""

        def insert_act_table_loads(self):
            import bass_rust as _bass_rust
            has_activation = any(
                isinstance(i, mybir.InstActivation)
                for blk in self.main_func.blocks
                for i in blk.instructions
            )
            if not has_activation:
                return
            tables = []
            for name, funcs in get_activation_tables(self.m.arch).items():
                if name == "natural_log_exp_and_others":
                    tables.append((name, funcs))
                else:
                    tables.append((name, set()))
            _bass_rust.insert_act_table_loads(self, tables)

    nc = _Bacc("TRN2", target_bir_lowering=False, debug=False,
               num_devices=N_CORES)
    h_t = nc.dram_tensor("h", [rows, IN_DIM], f32, kind="ExternalInput").ap()
    W_t = nc.dram_tensor("W", [ODIM, IN_DIM], f32, kind="ExternalInput").ap()
    b_t = nc.dram_tensor("b", [ODIM], f32, kind="ExternalInput").ap()
    out_t = nc.dram_tensor("out", [rows, NB], f32, kind="ExternalOutput").ap()
    with tile.TileContext(nc) as tc:
        with ExitStack() as ctx:
            build(ctx, tc, h_t, W_t, b_t, out_t, rows, G=G, reps=reps,
                  scatter_dup=scatter_dup, pair=pair, ilv=ilv,
                  es_bufs=es_bufs)
    nc.compile()
    return nc


_cache = {}


def kernel(h, W, b):
    h = np.ascontiguousarray(h, dtype=np.float32)
    W = np.ascontiguousarray(W, dtype=np.float32)
    b = np.ascontiguousarray(b, dtype=np.float32)
    rows = h.shape[0] // N_CORES
    key = ("nc", rows)
    if key not in _cache:
        _cache[key] = make_nc(rows)
    nc = _cache[key]
    from concourse.bass_utils import run_bass_kernel_spmd
    in_maps = [
        {"h": h[i * rows:(i + 1) * rows], "W": W, "b": b}
        for i in range(N_CORES)
    ]
    res = run_bass_kernel_spmd(nc, in_maps, core_ids=list(range(N_CORES)))
    return np.concatenate([r["out"] for r in res.results], axis=0)


if __name__ == "__main__":
    rng = np.random.default_rng(0)
    h = rng.standard_normal((B_FULL, IN_DIM), dtype=np.float32)
    W = (rng.standard_normal((ODIM, IN_DIM), dtype=np.float32) / 16.0)
    b = rng.standard_normal((ODIM,), dtype=np.float32) * 0.01
    out = kernel(h, W, b)
    print(out.shape, out.dtype, out[:2, :4])
